# revision 1
# baseline (speedup 1.0000x reference)
"""Multi-head self-attention on 8 TRN2 NeuronCores.

Sharding: core c -> (batch b = c//2, head-half g = c%2, i.e. 8 of 16 heads).
Each core computes qkv-proj + attention + out-proj partial for its 8 heads;
host sums the two partials per batch and adds b_out.

All large matmuls run as float32r (TF32-like, 1 cyc/row on the PE at N=512);
the attention-probabilities/V matmul runs in fp16. Softmax denominators come
from an appended ones column in V (row 64 of the ctx PSUM tile); the
normalization keeps every DVE write at partition base 0 (narrow DVE writes
at partition offset 64 corrupt partition 0 of unrelated SBUF tiles on HW)
and extracts the denominator row with a row-64 selector matmul.

Scores are computed per head pair with K=64 row-packed matmuls
(tile_position (0,0)/(64,0) derived from the operand partition bases); each
sk-chunk's pair of score tiles feeds one [128, 1024] Exp activation.
"""
import sys
sys.path.insert(0, '/opt/trn_rl_repo')

import numpy as np

import concourse.bass as bass
import concourse.mybir as mybir
import concourse.tile as tile
from concourse import bacc

F32 = mybir.dt.float32
F32R = mybir.dt.float32r
F16 = mybir.dt.float16

B, S, D = 4, 2048, 1024
H, HD = 16, 64            # total heads, head dim
HC = 8                    # heads per core
N_CORES = 8
SC = S // 512             # seq chunks of 512
EC = D // 128             # embed chunks of 128
NSK = S // 128            # sk chunks of 128


def build_nc(debug=False):
    nc = bacc.Bacc(None, target_bir_lowering=False)

    xT = nc.dram_tensor("xT", [D, S], F32R, kind="ExternalInput")
    w_qk = nc.dram_tensor("w_qk", [D, 1024], F32R, kind="ExternalInput")
    w_v = nc.dram_tensor("w_v", [D, 512], F32R, kind="ExternalInput")
    b_qk = nc.dram_tensor("b_qk", [128, 1024], F32R, kind="ExternalInput")
    b_v = nc.dram_tensor("b_v", [128, 512], F32R, kind="ExternalInput")
    onescol = nc.dram_tensor("onescol", [128, 128], F32R, kind="ExternalInput")
    sel = nc.dram_tensor("sel", [128, 64], F32R, kind="ExternalInput")
    ones = nc.dram_tensor("ones", [128, 512], F32R, kind="ExternalInput")
    w_out = nc.dram_tensor("w_out", [512, D], F32R, kind="ExternalInput")
    out = nc.dram_tensor("out", [S, D], F32, kind="ExternalOutput")
    if debug:
        dbg_denom = nc.dram_tensor("dbg_denom", [2, 16, 512], F32, kind="ExternalOutput")

    with tile.TileContext(nc) as tc:
        with (
            tc.tile_pool(name="const", bufs=1) as cpool,
            tc.tile_pool(name="pjt", bufs=2) as pjt_pool,
            tc.tile_pool(name="vpool", bufs=1) as vpool,
            tc.tile_pool(name="outsb", bufs=2) as out_pool,
            tc.tile_pool(name="s1x", bufs=2) as xpool,
            tc.tile_pool(name="s1wq", bufs=2) as wq_pool,
            tc.tile_pool(name="s1ps", bufs=1, space="PSUM") as s1ps,
        ):
            sel_sb = cpool.tile([128, 64], F32R)
            nc.sync.dma_start(sel_sb[:], sel[:])
            w_out_sb = cpool.tile([128, 4, D], F32R)
            nc.sync.dma_start(w_out_sb[:], w_out.rearrange("(c p) e -> p c e", p=128))
            ones_sb = cpool.tile([128, 512], F32R)
            nc.sync.dma_start(ones_sb[:], ones[:])
            b_qk_sb = cpool.tile([128, 1024], F32R)
            nc.sync.dma_start(b_qk_sb[:], b_qk[:])

            # v_sb[s % 128, s_tile, head, 0:64] = V; [..., 64] = 1.0
            v_sb = vpool.tile([128, S // 128, HC, HD + 1], F16)
            nc.vector.memset(v_sb[:, :, :, HD], 1.0)

            def stage1_pair_start(p):
                """Allocate tiles + weight DMA for pair p's q,k projection."""
                wq = wq_pool.tile([128, EC, 256], F32R, name="wq", tag="wq")
                # columns 0:128 = q of pair p, 128:256 = k of pair p
                wsrc = w_qk.rearrange("(c p) f -> p c f", p=128)
                nc.sync.dma_start(wq[:, :, 0:128],
                                  wsrc[:, :, 128 * p:128 * (p + 1)])
                nc.sync.dma_start(wq[:, :, 128:256],
                                  wsrc[:, :, 512 + 128 * p:512 + 128 * (p + 1)])
                pjt = pjt_pool.tile([128, 2, S], F32R, name="pjt", tag="pjt")
                return (pjt, wq)

            def stage1_pair_chunk(p, st, n, pool=None):
                """q,k projection of pair p for seq chunk n -> pjt[:, :, n]."""
                pjt, wq = st
                pool = pool or s1ps
                xt = xpool.tile([128, EC, 512], F32R, name="xtq", tag="xt")
                nc.sync.dma_start(
                    xt[:],
                    xT.rearrange("(c p) s -> p c s", p=128)[:, :, 512 * n:512 * (n + 1)],
                )
                for j in range(2):     # 0 = q, 1 = k
                    ps = pool.tile([128, 512], F32, name="qkps", tag="s1")
                    for ci in range(EC):
                        nc.tensor.matmul(
                            ps[:], wq[:, ci, 128 * j:128 * (j + 1)],
                            xt[:, ci, :], start=(ci == 0), stop=False)
                    boff = 512 * j + 128 * p
                    nc.tensor.matmul(ps[:], b_qk_sb[:, boff:boff + 128],
                                     ones_sb[:], start=False, stop=True)
                    nc.vector.tensor_copy(pjt[:, j, 512 * n:512 * (n + 1)], ps[:])

            def stage1_pair(p, pool=None):
                st = stage1_pair_start(p)
                for n in range(SC):
                    stage1_pair_chunk(p, st, n, pool)
                return st

            # ---- V projection (all heads) + pair-0 projection, with a
            # multi-bank prologue PSUM pool that closes before the attention
            # pools open ----
            with tc.tile_pool(name="s1wv", bufs=1) as wv_pool:
                b_v_sb = wv_pool.tile([128, 512], F32R)
                nc.sync.dma_start(b_v_sb[:], b_v[:])
                onescol_sb = wv_pool.tile([128, 128], F32R)
                nc.sync.dma_start(onescol_sb[:], onescol[:])
                w_v_sb = wv_pool.tile([128, EC, 512], F32R)
                nc.sync.dma_start(w_v_sb[:], w_v.rearrange("(c p) f -> p c f", p=128))
                for n in range(SC):
                    xt = xpool.tile([128, EC, 512], F32R, name="xtv", tag="xt")
                    nc.sync.dma_start(
                        xt[:],
                        xT.rearrange("(c p) s -> p c s", p=128)[:, :, 512 * n:512 * (n + 1)],
                    )
                    for tl in range(4):
                        t = 4 * n + tl
                        ps = s1ps.tile([128, 512], F32, name="vps", tag="s1")
                        for ci in range(EC):
                            nc.tensor.matmul(
                                ps[:], xt[:, ci, 128 * tl:128 * (tl + 1)],
                                w_v_sb[:, ci, :], start=(ci == 0), stop=False)
                        nc.tensor.matmul(ps[:], onescol_sb[:], b_v_sb[:],
                                         start=False, stop=True)
                        nc.vector.tensor_copy(
                            v_sb[:, t, :, 0:HD],
                            ps.rearrange("p (h d) -> p h d", h=HC))

            # ---- attention, stage1 of pair p+1 overlapped under pair p ----
            with (
                tc.tile_pool(name="expT", bufs=2) as expT_pool,
                tc.tile_pool(name="dnp", bufs=1) as dn_pool,
                tc.tile_pool(name="rcp", bufs=1) as rc_pool,
                tc.tile_pool(name="ctxT", bufs=1) as ctxT_pool,
                tc.tile_pool(name="scps", bufs=2, space="PSUM") as sc_ps,
                tc.tile_pool(name="ctxps", bufs=2, space="PSUM") as ctx_ps,
                tc.tile_pool(name="bcps", bufs=1, space="PSUM") as bc_ps,
            ):
                ctxT = ctxT_pool.tile([128, 4, S], F32R)
                dns = [dn_pool.tile([128, 512], F32, name=f"dn{i}")
                       for i in range(2)]
                for i in range(2):
                    nc.sync.dma_start(dns[i][:], ones.bitcast(F32)[:])

                pjts = {0: stage1_pair(0)}
                for p in range(4):
                    pjt = pjts.pop(p)[0]
                    for qc in range(SC):
                        qsl = slice(512 * qc, 512 * (qc + 1))
                        expTs = {}
                        ctxps = {}
                        for hi in range(2):
                            expTs[hi] = expT_pool.tile([128, NSK, 512], F16,
                                                       name=f"expT{hi}", tag="expT")
                            ctxps[hi] = ctx_ps.tile([HD + 1, 512], F32,
                                                    name=f"ctx{hi}", tag="ctx")

                        def stage3(grp):
                            for hi in range(2):
                                for gg in range(2):
                                    sk = 2 * grp + gg
                                    nc.tensor.matmul(
                                        ctxps[hi][:],
                                        v_sb[:, sk, 2 * p + hi, :],
                                        expTs[hi][:, sk, :],
                                        start=(sk == 0), stop=(sk == NSK - 1))

                        for grp in range(NSK // 2):
                            for hi in range(2):
                                base = 64 * hi
                                scp = sc_ps.tile([128, 2, 512], F32, name="scp")
                                for gg in range(2):
                                    sk = 2 * grp + gg
                                    nc.tensor.matmul(
                                        scp[:, gg, :],
                                        pjt[base:base + 64, 1,
                                            128 * sk:128 * (sk + 1)],
                                        pjt[base:base + 64, 0, qsl],
                                        start=True, stop=True)
                                nc.scalar.activation(
                                    expTs[hi][:, 2 * grp:2 * grp + 2, :],
                                    scp[:],
                                    mybir.ActivationFunctionType.Exp)
                            if grp > 0:
                                stage3(grp - 1)
                        stage3(NSK // 2 - 1)

                        # overlap next pair's projection under this pair's
                        # ACT-bound attention, one seq chunk per q-chunk
                        if p < 3:
                            if qc == 0:
                                pjts[p + 1] = stage1_pair_start(p + 1)
                            stage1_pair_chunk(p + 1, pjts[p + 1], qc)

                        # normalization: row 64 of ctx psum = sum(exp).
                        # All DVE writes stay at partition base 0; the
                        # row-64 selector matmul extracts the denominator.
                        for hi in range(2):
                            dn = dns[hi]
                            nc.vector.tensor_copy(dn[0:65, :], ctxps[hi][:])
                            nc.vector.memset(dn[0:64, :], 1.0)
                            rcf = rc_pool.tile([128, 512], F32,
                                               name=f"rcf{hi}", tag="rcf")
                            nc.vector.reciprocal_approx_fast(rcf[:], dn[:])
                            if debug:
                                nc.sync.dma_start(
                                    dbg_denom[hi:hi + 1, 4 * p + qc, :],
                                    dn[64:65, :])
                            rcp = rc_pool.tile([128, 512], F32R,
                                               name=f"rcp{hi}", tag="rcp")
                            nc.vector.tensor_copy(rcp[:], rcf[:])
                            bc = bc_ps.tile([64, 512], F32, name="bc", tag="bc")
                            nc.tensor.matmul(bc[:], sel_sb[:], rcp[:],
                                             start=True, stop=True)
                            bc_sb = rc_pool.tile([64, 512], F32, name="bcs",
                                                 tag="bcs")
                            nc.vector.tensor_copy(bc_sb[:], bc[:])
                            nc.vector.tensor_mul(
                                ctxT[64 * hi:64 * (hi + 1), p, qsl],
                                ctxps[hi][0:64, :], bc_sb[:])

                        # stage 4 for this q-chunk once the last pair's
                        # ctxT columns are normalized
                        if p == 3:
                            for tl in range(4):
                                tq = 4 * qc + tl
                                for ec in range(2):
                                    ps4 = bc_ps.tile([128, 512], F32,
                                                     name="s4", tag="bc")
                                    for pp in range(4):
                                        nc.tensor.matmul(
                                            ps4[:],
                                            ctxT[:, pp, 128 * tq:128 * (tq + 1)],
                                            w_out_sb[:, pp, 512 * ec:512 * (ec + 1)],
                                            start=(pp == 0), stop=(pp == 3))
                                    o = out_pool.tile([128, 512], F32, name="o")
                                    nc.vector.tensor_copy(o[:], ps4[:])
                                    nc.sync.dma_start(
                                        out[128 * tq:128 * (tq + 1),
                                            512 * ec:512 * (ec + 1)], o[:])



    nc.compile()
    return nc


# ---------------------------------------------------------------------------
# host side: shard, run SPMD, gather
# ---------------------------------------------------------------------------

_RUNNER = None


def _make_runner(nc, n_cores):
    """Jit-once SPMD runner via PJRT (axon)."""
    import jax
    from jax.sharding import Mesh, PartitionSpec
    from jax.experimental.shard_map import shard_map
    from concourse import bass2jax
    from concourse.bass2jax import _bass_exec_p, install_neuronx_cc_hook

    install_neuronx_cc_hook()
    partition_name = nc.partition_id_tensor.name if nc.partition_id_tensor else None

    in_names, out_names, out_avals, zero_outs = [], [], [], []
    for alloc in nc.m.functions[0].allocations:
        if not isinstance(alloc, mybir.MemoryLocationSet):
            continue
        name = alloc.memorylocations[0].name
        if alloc.kind == "ExternalInput":
            if name != partition_name:
                in_names.append(name)
        elif alloc.kind == "ExternalOutput":
            out_names.append(name)
            shape = tuple(alloc.tensor_shape)
            dtype = mybir.dt.np(alloc.dtype)
            out_avals.append(jax.core.ShapedArray(shape, dtype))
            zero_outs.append(np.zeros(shape, dtype))
    n_params = len(in_names)
    n_outs = len(out_avals)
    all_in_names = list(in_names) + list(out_names)
    if partition_name is not None:
        all_in_names.append(partition_name)

    def _body(*args):
        operands = list(args)
        if partition_name is not None:
            operands.append(bass2jax.partition_id_tensor())
        outs = _bass_exec_p.bind(
            *operands,
            out_avals=tuple(out_avals),
            in_names=tuple(all_in_names),
            out_names=tuple(out_names),
            lowering_input_output_aliases=(),
            sim_require_finite=True,
            sim_require_nnan=True,
            nc=nc,
        )
        return tuple(outs)

    devices = jax.devices()[:n_cores]
    if n_cores == 1:
        jitted = jax.jit(_body, keep_unused=True)

        def run1(in_maps):
            args = [np.asarray(in_maps[0][n]) for n in in_names] + list(zero_outs)
            out_arrs = jitted(*args)
            jax.block_until_ready(out_arrs)
            return [{n: np.asarray(out_arrs[i]) for i, n in enumerate(out_names)}]

        return run1

    mesh = Mesh(np.asarray(devices), ("core",))
    in_specs = (PartitionSpec("core"),) * (n_params + n_outs)
    out_specs = (PartitionSpec("core"),) * n_outs
    jitted = jax.jit(
        shard_map(_body, mesh=mesh, in_specs=in_specs, out_specs=out_specs,
                  check_rep=False),
        keep_unused=True,
    )

    def run(in_maps):
        concat_in = [
            np.concatenate([np.asarray(in_maps[c][n]) for c in range(n_cores)],
                           axis=0)
            for n in in_names
        ]
        concat_zero = [
            np.zeros((n_cores * z.shape[0], *z.shape[1:]), z.dtype)
            for z in zero_outs
        ]
        out_arrs = jitted(*concat_in, *concat_zero)
        jax.block_until_ready(out_arrs)
        return [
            {n: np.asarray(out_arrs[i]).reshape(n_cores, *out_avals[i].shape)[c]
             for i, n in enumerate(out_names)}
            for c in range(n_cores)
        ]

    return run


def _shard_inputs(qkv, W_in, b_in, W_out, b_out):
    """Build the 8 per-core input dicts."""
    x = np.asarray(qkv, np.float32)
    W_in = np.asarray(W_in, np.float32)
    b_in = np.asarray(b_in, np.float32)
    W_out = np.asarray(W_out, np.float32)
    scale = np.float32(1.0 / np.sqrt(HD))
    ones = np.ones((128, 512), np.float32)
    onescol = np.zeros((128, 128), np.float32)
    onescol[0, :] = 1.0
    sel = np.zeros((128, 64), np.float32)
    sel[64, :] = 1.0

    in_maps = []
    for c in range(N_CORES):
        b, g = divmod(c, 2)
        qs = slice(512 * g, 512 * (g + 1))
        ks = slice(1024 + 512 * g, 1024 + 512 * (g + 1))
        vs = slice(2048 + 512 * g, 2048 + 512 * (g + 1))
        b_qk_pad = np.zeros((128, 1024), np.float32)
        b_qk_pad[0, :] = np.concatenate([b_in[qs] * scale, b_in[ks]])
        b_v_pad = np.zeros((128, 512), np.float32)
        b_v_pad[0, :] = b_in[vs]
        in_maps.append({
            "xT": np.ascontiguousarray(x[b].T),
            "w_qk": np.ascontiguousarray(
                np.concatenate([W_in[:, qs] * scale, W_in[:, ks]], axis=1)),
            "w_v": np.ascontiguousarray(W_in[:, vs]),
            "b_qk": b_qk_pad,
            "b_v": b_v_pad,
            "onescol": onescol,
            "sel": sel,
            "ones": ones,
            "w_out": np.ascontiguousarray(W_out[512 * g:512 * (g + 1), :]),
        })
    return in_maps


def kernel(qkv, W_in, b_in, W_out, b_out):
    global _RUNNER
    if _RUNNER is None:
        nc = build_nc()
        _RUNNER = _make_runner(nc, N_CORES)
    in_maps = _shard_inputs(qkv, W_in, b_in, W_out, b_out)
    results = _RUNNER(in_maps)
    b_out = np.asarray(b_out, np.float32)
    out = np.empty((B, S, D), np.float32)
    for b in range(B):
        out[b] = results[2 * b]["out"] + results[2 * b + 1]["out"] + b_out
    return out


if __name__ == "__main__":
    rng = np.random.default_rng(0)
    qkv = rng.standard_normal((B, S, D)).astype(np.float32)
    sc = 1.0 / np.sqrt(D)
    W_in = rng.uniform(-sc, sc, (D, 3 * D)).astype(np.float32)
    b_in = rng.uniform(-sc, sc, (3 * D,)).astype(np.float32)
    W_out = rng.uniform(-sc, sc, (D, D)).astype(np.float32)
    b_out = rng.uniform(-sc, sc, (D,)).astype(np.float32)
    got = kernel(qkv, W_in, b_in, W_out, b_out)
    print("kernel ran, output shape", got.shape)



# revision 3
# speedup vs baseline: 1.2906x; 1.2906x over previous
"""Multi-head self-attention on 8 TRN2 NeuronCores — v2.

Sharding: core c -> (batch b = c//2, head-half g = c%2, heads 8g..8g+7).
Each core computes qkv-proj + attention + out-proj partial for its 8 heads;
host sums the two partials per batch and adds the output bias.

Stage 1 (qkv projections) runs as fp8e4m3 DoubleRow matmuls (K=256 per
instruction at 0.5 cyc/row): x and the weights are host-folded to a
[128, 2, 4, *] layout with embed index e = 256*gk + 128*i + p. The q/k
biases are added exactly during the PSUM->SBUF copy as per-partition
tensor_scalar adds; the V bias is folded into the host-side output bias
(softmax rows sum to 1, so ctx = ctx_nobias + b_v, and b_v @ W_out is a
constant added on the host).

Scores run fp16 per head pair (K=64 row-packed via partition bases).
Exp on ACT (the overall bottleneck) writes fp8 probabilities with the two
sk-chunks of each score tile side by side; PV then runs fp8 DoubleRow
with those two chunks as the two K-slots, producing ctx TRANSPOSED
([q, d] with q on all 128 partitions) plus an appended ones-column that
accumulates the softmax denominator per q row. Normalization is a
per-partition reciprocal+scale on DVE, then a PE fp16 transpose restores
[d, q] for the fp16 out-projection.
"""
import sys
sys.path.insert(0, '/opt/trn_rl_repo')

import numpy as np
import ml_dtypes

import concourse.bass as bass
import concourse.mybir as mybir
import concourse.tile as tile
from concourse import bacc

F32 = mybir.dt.float32
F16 = mybir.dt.float16
F8 = mybir.dt.float8e4
E8NP = ml_dtypes.float8_e4m3
DR = mybir.MatmulPerfMode.DoubleRow
EXP = mybir.ActivationFunctionType.Exp

B, S, D = 4, 2048, 1024
H, HD = 16, 64            # total heads, head dim
HC = 8                    # heads per core
N_CORES = 8
SC = S // 512             # seq chunks of 512
NSK = S // 128            # sk chunks of 128
NG = NSK // 2             # DoubleRow groups of 2 sk chunks


def build_nc(debug=False):
    nc = bacc.Bacc(None, target_bir_lowering=False)

    x8 = nc.dram_tensor("x8", [128, 2, 4, S], F8, kind="ExternalInput")
    w_qk8 = nc.dram_tensor("w_qk8", [128, 2, 4, 1024], F8, kind="ExternalInput")
    w_v8 = nc.dram_tensor("w_v8", [128, 2, 4, 512], F8, kind="ExternalInput")
    b_qk = nc.dram_tensor("b_qk", [128, 4, 2], F32, kind="ExternalInput")
    ident = nc.dram_tensor("ident", [128, 128], F16, kind="ExternalInput")
    w_out = nc.dram_tensor("w_out", [128, 4, 1024], F16, kind="ExternalInput")
    out = nc.dram_tensor("out", [S, D], F32, kind="ExternalOutput")

    with tile.TileContext(nc) as tc:
        with (
            tc.tile_pool(name="const", bufs=1) as cpool,
            tc.tile_pool(name="pjt", bufs=2) as pjt_pool,
            tc.tile_pool(name="vpool", bufs=1) as vpool,
            tc.tile_pool(name="expT", bufs=2) as expT_pool,
            tc.tile_pool(name="ctxTp", bufs=1) as ctxT_pool,
            tc.tile_pool(name="ctxs", bufs=3) as ctx_pool,
            tc.tile_pool(name="rcpp", bufs=3) as rcp_pool,
            tc.tile_pool(name="outsb", bufs=2) as out_pool,
        ):
            x8_sb = cpool.tile([128, 2, 4, S], F8)
            nc.sync.dma_start(x8_sb[:], x8[:])
            w_qk_sb = cpool.tile([128, 2, 4, 1024], F8)
            nc.sync.dma_start(w_qk_sb[:], w_qk8[:])
            b_qk_sb = cpool.tile([128, 4, 2], F32)
            nc.sync.dma_start(b_qk_sb[:], b_qk[:])
            ident_sb = cpool.tile([128, 128], F16)
            nc.sync.dma_start(ident_sb[:], ident[:])
            w_out_sb = cpool.tile([128, 4, 1024], F16)
            nc.sync.dma_start(w_out_sb[:], w_out[:])

            # v8[s % 128, s_tile, head, 0:64] = V (no bias); [..., 64] = 1.0
            v8 = vpool.tile([128, NSK, HC, HD + 1], F8)
            nc.vector.memset(v8[:, :, :, HD], 1.0)
            ctxT = ctxT_pool.tile([128, 4, S], F16)

            def s1_pair_chunk(p, pjt, n, pool):
                """q,k projection of pair p for seq chunk n -> pjt[:, :, n]."""
                for j in range(2):     # 0 = q, 1 = k
                    ps = pool.tile([128, 512], F32, name="s1ps", tag="s1op")
                    cs = 512 * j + 128 * p
                    for gk in range(4):
                        nc.tensor.matmul(
                            ps[:], w_qk_sb[:, :, gk, cs:cs + 128],
                            x8_sb[:, :, gk, 512 * n:512 * (n + 1)],
                            start=(gk == 0), stop=(gk == 3), perf_mode=DR)
                    nc.vector.tensor_scalar_add(
                        pjt[:, j, 512 * n:512 * (n + 1)], ps[:],
                        b_qk_sb[:, p, j:j + 1])

            # ---- prologue: pair-0 q,k projection + V projection (all heads)
            # in a multi-bank PSUM pool that closes before attention opens ----
            pjt0 = pjt_pool.tile([128, 2, S], F16, name="pjt", tag="pjt")
            with tc.tile_pool(name="s1wv", bufs=1) as wv_pool, \
                 tc.tile_pool(name="pro_ps", bufs=2, space="PSUM") as pro_ps:
                w_v_sb = wv_pool.tile([128, 2, 4, 512], F8)
                nc.sync.dma_start(w_v_sb[:], w_v8[:])
                s1_pair_chunk(0, pjt0, 0, pro_ps)
                for n in range(SC):
                    for tl in range(4):
                        t = 4 * n + tl
                        ps = pro_ps.tile([128, 512], F32, name="vps", tag="s1op")
                        for gk in range(4):
                            nc.tensor.matmul(
                                ps[:],
                                x8_sb[:, :, gk, 128 * t:128 * (t + 1)],
                                w_v_sb[:, :, gk, :],
                                start=(gk == 0), stop=(gk == 3), perf_mode=DR)
                        nc.vector.tensor_copy(
                            v8[:, t, :, 0:HD],
                            ps.rearrange("p (h d) -> p h d", h=HC))
                for n in range(1, SC):
                    s1_pair_chunk(0, pjt0, n, pro_ps)

            # ---- attention ----
            with (
                tc.tile_pool(name="scps", bufs=2, space="PSUM") as sc_ps,
                tc.tile_pool(name="pvps", bufs=2, space="PSUM") as pv_ps,
                tc.tile_pool(name="tpps", bufs=1, space="PSUM") as tp_ps,
                tc.tile_pool(name="s1op", bufs=1, space="PSUM") as s1op_ps,
            ):
                pjts = {0: pjt0}
                for p in range(4):
                    pjt = pjts.pop(p)
                    for qc in range(SC):
                        qsl = slice(512 * qc, 512 * (qc + 1))
                        expT = expT_pool.tile([128, NSK, 2, 512], F8,
                                              name="expT", tag="expT")
                        for hi in range(2):
                            base = 64 * hi
                            for grp in range(NG):
                                scp = sc_ps.tile([128, 2, 512], F32,
                                                 name="scp", tag="scp")
                                for gg in range(2):
                                    sk = 2 * grp + gg
                                    nc.tensor.matmul(
                                        scp[:, gg, :],
                                        pjt[base:base + 64, 1,
                                            128 * sk:128 * (sk + 1)],
                                        pjt[base:base + 64, 0, qsl],
                                        start=True, stop=True)
                                nc.scalar.activation(
                                    expT[:, 2 * grp:2 * grp + 2, hi, :],
                                    scp[:], EXP)

                        # overlap next pair's projection under the ACT-bound
                        # attention, one seq chunk per q-chunk
                        if p < 3:
                            if qc == 0:
                                pjts[p + 1] = pjt_pool.tile(
                                    [128, 2, S], F16, name="pjt", tag="pjt")
                            s1_pair_chunk(p + 1, pjts[p + 1], qc, s1op_ps)

                        # PV (fp8 DoubleRow) + normalize + transpose
                        for hi in range(2):
                            h = 2 * p + hi
                            for jq in range(4):
                                qb = 512 * qc + 128 * jq
                                pv = pv_ps.tile([128, 512], F32,
                                                name="pv", tag="pv")
                                for grp in range(NG):
                                    nc.tensor.matmul(
                                        pv[:, 0:HD + 1],
                                        expT[:, 2 * grp:2 * grp + 2, hi,
                                             128 * jq:128 * (jq + 1)],
                                        v8[:, 2 * grp:2 * grp + 2, h, :],
                                        start=(grp == 0), stop=(grp == NG - 1),
                                        perf_mode=DR)
                                rcp = rcp_pool.tile([128, 1], F32, name="rcp",
                                                    tag="rcp")
                                nc.vector.reciprocal(rcp[:], pv[:, HD:HD + 1])
                                cx = ctx_pool.tile([128, HD], F16, name="cx",
                                                   tag="cx")
                                nc.vector.tensor_scalar_mul(
                                    cx[:], pv[:, 0:HD], rcp[:])
                                tp = tp_ps.tile([64, 1024], F16, name="tp",
                                                tag="tp")
                                nc.tensor.transpose(
                                    tp[:, 0:128], cx[:], ident_sb[:])
                                nc.vector.tensor_copy(
                                    ctxT[base:base + 64, p, qb:qb + 128],
                                    tp[:, 0:128])

                        # out-projection for this q-chunk once the last
                        # pair's ctxT columns are in place
                        if p == 3:
                            for tl in range(4):
                                tq = 4 * qc + tl
                                for ec in range(2):
                                    ps4 = s1op_ps.tile([128, 512], F32,
                                                       name="s4", tag="s1op")
                                    for pp in range(4):
                                        nc.tensor.matmul(
                                            ps4[:],
                                            ctxT[:, pp,
                                                 128 * tq:128 * (tq + 1)],
                                            w_out_sb[:, pp,
                                                     512 * ec:512 * (ec + 1)],
                                            start=(pp == 0), stop=(pp == 3))
                                    o = out_pool.tile([128, 512], F32,
                                                      name="o")
                                    nc.vector.tensor_copy(o[:], ps4[:])
                                    nc.sync.dma_start(
                                        out[128 * tq:128 * (tq + 1),
                                            512 * ec:512 * (ec + 1)], o[:])

    nc.compile()
    return nc


# ---------------------------------------------------------------------------
# host side: shard, run SPMD, gather
# ---------------------------------------------------------------------------

_RUNNER = None


def _make_runner(nc, n_cores):
    """Jit-once SPMD runner via PJRT (axon)."""
    import jax
    from jax.sharding import Mesh, PartitionSpec
    from jax.experimental.shard_map import shard_map
    from concourse import bass2jax
    from concourse.bass2jax import _bass_exec_p, install_neuronx_cc_hook

    install_neuronx_cc_hook()
    partition_name = nc.partition_id_tensor.name if nc.partition_id_tensor else None

    in_names, out_names, out_avals, zero_outs = [], [], [], []
    for alloc in nc.m.functions[0].allocations:
        if not isinstance(alloc, mybir.MemoryLocationSet):
            continue
        name = alloc.memorylocations[0].name
        if alloc.kind == "ExternalInput":
            if name != partition_name:
                in_names.append(name)
        elif alloc.kind == "ExternalOutput":
            out_names.append(name)
            shape = tuple(alloc.tensor_shape)
            dtype = mybir.dt.np(alloc.dtype)
            out_avals.append(jax.core.ShapedArray(shape, dtype))
            zero_outs.append(np.zeros(shape, dtype))
    n_params = len(in_names)
    n_outs = len(out_avals)
    all_in_names = list(in_names) + list(out_names)
    if partition_name is not None:
        all_in_names.append(partition_name)

    def _body(*args):
        operands = list(args)
        if partition_name is not None:
            operands.append(bass2jax.partition_id_tensor())
        outs = _bass_exec_p.bind(
            *operands,
            out_avals=tuple(out_avals),
            in_names=tuple(all_in_names),
            out_names=tuple(out_names),
            lowering_input_output_aliases=(),
            sim_require_finite=True,
            sim_require_nnan=True,
            nc=nc,
        )
        return tuple(outs)

    devices = jax.devices()[:n_cores]
    if n_cores == 1:
        jitted = jax.jit(_body, keep_unused=True)

        def run1(in_maps):
            args = [np.asarray(in_maps[0][n]) for n in in_names] + list(zero_outs)
            out_arrs = jitted(*args)
            jax.block_until_ready(out_arrs)
            return [{n: np.asarray(out_arrs[i]) for i, n in enumerate(out_names)}]

        return run1

    mesh = Mesh(np.asarray(devices), ("core",))
    in_specs = (PartitionSpec("core"),) * (n_params + n_outs)
    out_specs = (PartitionSpec("core"),) * n_outs
    jitted = jax.jit(
        shard_map(_body, mesh=mesh, in_specs=in_specs, out_specs=out_specs,
                  check_rep=False),
        keep_unused=True,
    )

    def run(in_maps):
        concat_in = [
            np.concatenate([np.asarray(in_maps[c][n]) for c in range(n_cores)],
                           axis=0)
            for n in in_names
        ]
        concat_zero = [
            np.zeros((n_cores * z.shape[0], *z.shape[1:]), z.dtype)
            for z in zero_outs
        ]
        out_arrs = jitted(*concat_in, *concat_zero)
        jax.block_until_ready(out_arrs)
        return [
            {n: np.asarray(out_arrs[i]).reshape(n_cores, *out_avals[i].shape)[c]
             for i, n in enumerate(out_names)}
            for c in range(n_cores)
        ]

    return run


def _fold(m):
    """[1024, C] -> [128, 2, 4, C] with row e = 256*gk + 128*i + p."""
    return np.ascontiguousarray(
        m.reshape(4, 2, 128, m.shape[1]).transpose(2, 1, 0, 3))


def _shard_inputs(qkv, W_in, b_in, W_out, b_out):
    """Build the 8 per-core input dicts."""
    x = np.asarray(qkv, np.float32)
    W_in = np.asarray(W_in, np.float32)
    b_in = np.asarray(b_in, np.float32)
    W_out = np.asarray(W_out, np.float32)
    scale = np.float32(1.0 / np.sqrt(HD))
    ident = np.eye(128, dtype=np.float16)

    in_maps = []
    for c in range(N_CORES):
        b, g = divmod(c, 2)
        qs = slice(512 * g, 512 * (g + 1))
        ks = slice(1024 + 512 * g, 1024 + 512 * (g + 1))
        vs = slice(2048 + 512 * g, 2048 + 512 * (g + 1))
        bq = (b_in[qs] * scale).reshape(4, 128).T
        bk = b_in[ks].reshape(4, 128).T
        in_maps.append({
            "x8": _fold(np.ascontiguousarray(x[b].T)).astype(E8NP),
            "w_qk8": _fold(np.concatenate(
                [W_in[:, qs] * scale, W_in[:, ks]], axis=1)).astype(E8NP),
            "w_v8": _fold(W_in[:, vs]).astype(E8NP),
            "b_qk": np.ascontiguousarray(
                np.stack([bq, bk], axis=-1)).astype(np.float32),
            "ident": ident,
            "w_out": np.ascontiguousarray(
                W_out[512 * g:512 * (g + 1)].reshape(4, 128, 1024)
                .transpose(1, 0, 2)).astype(np.float16),
        })
    return in_maps


def kernel(qkv, W_in, b_in, W_out, b_out):
    global _RUNNER
    if _RUNNER is None:
        nc = build_nc()
        _RUNNER = _make_runner(nc, N_CORES)
    in_maps = _shard_inputs(qkv, W_in, b_in, W_out, b_out)
    results = _RUNNER(in_maps)
    b_in = np.asarray(b_in, np.float32)
    W_out_f = np.asarray(W_out, np.float32)
    # V bias folded through the out-projection (softmax rows sum to 1)
    bias = np.asarray(b_out, np.float32) + b_in[2 * D:] @ W_out_f
    out = np.empty((B, S, D), np.float32)
    for b in range(B):
        out[b] = results[2 * b]["out"] + results[2 * b + 1]["out"] + bias
    return out


if __name__ == "__main__":
    rng = np.random.default_rng(0)
    qkv = rng.standard_normal((B, S, D)).astype(np.float32)
    sc = 1.0 / np.sqrt(D)
    W_in = rng.uniform(-sc, sc, (D, 3 * D)).astype(np.float32)
    b_in = rng.uniform(-sc, sc, (3 * D,)).astype(np.float32)
    W_out = rng.uniform(-sc, sc, (D, D)).astype(np.float32)
    b_out = rng.uniform(-sc, sc, (D,)).astype(np.float32)
    got = kernel(qkv, W_in, b_in, W_out, b_out)
    print("kernel ran, output shape", got.shape)
